# revision 1
# baseline (speedup 1.0000x reference)
"""Trainium2 Bass kernel for nn_ELM_AE_FatSpectral_Ensemble.

Data-parallel over batch: 4 samples/core on 8 cores. Per (sample, member):
  XrT[w,c] = RfullT.T @ x.T   (bilinear+antialias resize via PE matmul)
  XnT      = zscore(XrT) along c (bn_stats + per-partition scale on free dim)
  H        = sigmoid(W @ Xn);  G = H H^T (16x16)
  G^-1 via Newton-Schulz on block-diagonal [128,128] supermatrices
  var(B), B = C G^-1, C = Xn H^T:
     quad_i = C_i G^-2 C_i^T, t_i = C_i (G^-1 1)
     out_i = quad_i/(Q-1) - t_i^2/(Q(Q-1))
Blocks of the supermatrix: b = 4*si + m (si = sample within pair), one
supermatrix per sample pair (spr), built super-outer so Newton-Schulz of
super 0 overlaps phase 1 of super 1. Partition-offset matmuls avoided via
full-K (128) matmuls with zero-masked stationary operands.
"""

import numpy as np

import concourse.bacc as bacc
import concourse.tile as tile
from concourse import mybir
from concourse.bass_utils import run_bass_kernel_spmd

F32 = mybir.dt.float32
F32R = mybir.dt.float32r
AF = mybir.ActivationFunctionType
ALU = mybir.AluOpType

S = 4
NCORES = 8
SP = 14
WH = SP * SP
Q = 16
MEMBERS = [(256, 56), (512, 28), (1024, 14), (2048, 7)]
OFFS = [0, 256, 768, 1792]
DTOT = 3840
NEWTON_ITERS = 10
GSZ = {0: 1, 1: 2, 2: 4, 3: 16}      # c-chunks per input DMA (~1.6MB each)


def _weight_mat(n_in, n_out):
    scale = n_out / n_in
    kernel_scale = max(1.0, 1.0 / scale)
    sample_f = (np.arange(n_out) + 0.5) / scale - 0.5
    x = np.abs(sample_f[:, None] - np.arange(n_in)[None, :]) / kernel_scale
    w = np.maximum(0.0, 1.0 - x)
    total = w.sum(axis=1, keepdims=True)
    return (w / np.where(total > 0, total, 1)).astype(np.float32)


def _consts():
    rts = {}
    for m, (c, sp) in enumerate(MEMBERS):
        if sp == SP:
            continue
        R = _weight_mat(sp, SP)
        rt = np.kron(R, R).T.astype(np.float32)   # [uv, 196]
        pad = (-rt.shape[0]) % 128
        if pad:
            rt = np.concatenate([rt, np.zeros((pad, WH), np.float32)], 0)
        rts[m] = np.ascontiguousarray(rt)
    ident = np.eye(128, dtype=np.float32)
    p16 = np.kron(np.eye(8, dtype=np.float32), np.ones((16, 16), np.float32))
    p16 = p16.astype(np.float32)
    # mask8[m, spr][p, col]: per-super sample masks
    mask8 = np.zeros((4, 2, 128, 4), np.float32)
    for m in range(4):
        for spr in range(2):
            for si in range(2):
                b = 4 * si + m
                mask8[m, spr, 16 * b:16 * b + 16, 2 * spr + si] = 1.0
    mask8 = np.ascontiguousarray(mask8.transpose(2, 0, 1, 3))  # [128, 4, 2, 4]
    return rts, ident, p16, mask8


def _chunks(n, sz=128):
    return [(i, min(sz, n - i)) for i in range(0, n, sz)]


def _build_program(debug=False):
    rts, ident_np, p16_np, mask8_np = _consts()

    nc = bacc.Bacc()
    xin, wt, rtd = {}, {}, {}
    for m, (c, sp) in enumerate(MEMBERS):
        xin[m] = nc.dram_tensor(f"x{m}", [S, c, sp * sp], F32R, kind="ExternalInput")
        wt[m] = nc.dram_tensor(f"wt{m}", [c, Q], F32R, kind="ExternalInput")
        if m in rts:
            rtd[m] = nc.dram_tensor(f"rt{m}", list(rts[m].shape), F32R,
                                    kind="ExternalInput")
    identd = nc.dram_tensor("ident", [128, 128], F32, kind="ExternalInput")
    p16d = nc.dram_tensor("p16", [128, 128], F32, kind="ExternalInput")
    mask8d = nc.dram_tensor("mask8", [128, 4, 2, 4], F32, kind="ExternalInput")
    outd = nc.dram_tensor("out", [S, DTOT], F32, kind="ExternalOutput")

    nzk = {}
    for m in rts:
        uv = MEMBERS[m][1] ** 2
        nzk[m] = {}
        for Mi, (Mo, Msz) in enumerate([(0, 128), (128, 68)]):
            nzk[m][Mi] = [ki for ki, (ko, ksz) in enumerate(_chunks(uv))
                          if np.any(rts[m][ko:ko + ksz, Mo:Mo + Msz] != 0)]

    from contextlib import ExitStack
    _ceng = [0]
    _deng = [0]

    def _pcopy(out, in_):
        _ceng[0] ^= 1
        (nc.scalar.copy if _ceng[0] else nc.vector.tensor_copy)(out=out, in_=in_)

    def _dma(out, in_):
        _deng[0] ^= 1
        (nc.sync if _deng[0] else nc.gpsimd).dma_start(out=out, in_=in_)

    with tile.TileContext(nc) as tc, ExitStack() as _es:
        _p = lambda **kw: _es.enter_context(tc.tile_pool(**kw))
        consts = _p(name="consts", bufs=1)
        xinp = _p(name="xinp", bufs=2)
        xtp = _p(name="xtp", bufs=1)
        xntp = _p(name="xntp", bufs=2)
        xnp = _p(name="xnp", bufs=3)
        hp = _p(name="hp", bufs=2)
        htp = _p(name="htp", bufs=2)
        sup = _p(name="sup", bufs=1)
        newt = _p(name="newt", bufs=2)
        smalls = _p(name="smalls", bufs=4)
        psbp = _p(name="psbp", bufs=2)
        tsqp = _p(name="tsqp", bufs=2)
        outp = _p(name="outp", bufs=2)
        pt = _p(name="pt", bufs=2, space="PSUM")      # 2-bank tiles
        pacc = _p(name="pacc", bufs=2, space="PSUM")
        hq = _p(name="hq", bufs=2, space="PSUM")
        if True:
            # ---------- constants ----------
            ident_sb = consts.tile([128, 128], F32, tag="ident")
            nc.gpsimd.dma_start(out=ident_sb, in_=identd[:, :])
            identr_sb = consts.tile([128, 128], F32R, tag="identr")
            nc.gpsimd.dma_start(out=identr_sb, in_=identd[:, :].bitcast(F32R))
            p16_sb = consts.tile([128, 128], F32, tag="p16")
            nc.gpsimd.dma_start(out=p16_sb, in_=p16d[:, :])
            mask8_sb = consts.tile([128, 4, 2, 4], F32, tag="mask8")
            nc.gpsimd.dma_start(out=mask8_sb, in_=mask8d[:, :, :, :])
            oq8_sb = consts.tile([128, 4, 2, 4], F32, tag="oq8")
            nc.vector.tensor_scalar_mul(
                oq8_sb.rearrange("p a b c -> p (a b c)").bitcast(F32R),
                mask8_sb.rearrange("p a b c -> p (a b c)"), 1.0 / (Q - 1))
            ones_sb = consts.tile([128, 1], F32, tag="ones")
            nc.vector.memset(ones_sb, 1.0)
            zer_sb = consts.tile([128, 256], F32, tag="zer")
            nc.vector.memset(zer_sb, 0.0)

            rtt, wtt = {}, {}
            for m, (c, sp) in enumerate(MEMBERS):
                cc_n = c // 128
                wtt[m] = consts.tile([128, cc_n, Q], F32R, tag=f"wt{m}",
                                     name=f"wt{m}")
                nc.gpsimd.dma_start(
                    out=wtt[m],
                    in_=wt[m][:, :].rearrange("(k p) q -> p k q", p=128))
                if m in rts:
                    nk = rts[m].shape[0] // 128
                    rtt[m] = consts.tile([128, nk, WH], F32R, tag=f"rt{m}",
                                         name=f"rt{m}")
                    nc.gpsimd.dma_start(
                        out=rtt[m],
                        in_=rtd[m][:, :].rearrange("(k p) w -> p k w", p=128))

            ct_all, gsup = [], []
            for spr in range(2):
                t = sup.tile([128, DTOT], F32, tag=f"ct{spr}", name=f"ct{spr}")
                ct_all.append(t)
                g = sup.tile([128, 128], F32, tag=f"gsup{spr}",
                             name=f"gsup{spr}")
                nc.vector.memset(g, 0.0)
                gsup.append(g)

            m2_sb = [None, None]
            r_sb = [None, None]

            # ============ phase 1+2, super-outer =============
            for spr in range(2):
                for m, (c, sp) in enumerate(MEMBERS):
                    uv = sp * sp
                    cc_n = c // 128
                    kch = _chunks(uv)
                    nch = _chunks(c, 512)
                    gsz = GSZ[m]

                    xnt0, xnt1 = {}, {}
                    for si in range(2):
                        s = 2 * spr + si
                        x0t = xntp.tile([128, c], F32, tag=f"xnt0_{si}",
                                        name=f"xnt0_{si}")
                        x1t = xntp.tile([68, c], F32, tag=f"xnt1_{si}",
                                        name=f"xnt1_{si}")
                        xnt0[si], xnt1[si] = x0t, x1t

                        if m == 2:
                            # identity resize: transpose straight into XrT
                            for g0 in range(0, cc_n, gsz):
                                gn = min(gsz, cc_n - g0)
                                xt_in = xinp.tile([128, gsz, uv], F32R,
                                                  tag="xin", name="xt_in")
                                _dma(
                                    out=xt_in[:, :gn, :],
                                    in_=xin[m][s, g0 * 128:(g0 + gn) * 128,
                                               :].rearrange(
                                        "(k p) v -> p k v", p=128))
                                for c2 in range(0, gn, 2):
                                    cn = min(2, gn - c2)
                                    ps = pt.tile([128, 1024], F32R, tag="pt")
                                    for ci in range(c2, c2 + cn):
                                        off = (ci - c2) * 256
                                        nc.tensor.transpose(
                                            ps[:128, off:off + 128],
                                            xt_in[:, ci, 0:128], identr_sb)
                                        nc.tensor.transpose(
                                            ps[:68, off + 128:off + 256],
                                            xt_in[:, ci, 128:196], identr_sb)
                                    for ci in range(c2, c2 + cn):
                                        cc = g0 + ci
                                        off = (ci - c2) * 256
                                        _pcopy(
                                            out=x0t[:, cc * 128:(cc + 1) *
                                                    128].bitcast(F32R),
                                            in_=ps[:128, off:off + 128])
                                        _pcopy(
                                            out=x1t[:68, cc * 128:(cc + 1) *
                                                    128].bitcast(F32R),
                                            in_=ps[:68, off + 128:off + 256])
                        else:
                            xt = xtp.tile([128, len(kch), c], F32R, tag="xt",
                                          name="xt")
                            for g0 in range(0, cc_n, gsz):
                                gn = min(gsz, cc_n - g0)
                                xt_in = xinp.tile([128, gsz, uv], F32R,
                                                  tag="xin", name="xt_in")
                                _dma(
                                    out=xt_in[:, :gn, :],
                                    in_=xin[m][s, g0 * 128:(g0 + gn) * 128,
                                               :].rearrange(
                                        "(k p) v -> p k v", p=128))
                                for ci in range(gn):
                                    cc = g0 + ci
                                    for kb in range(0, len(kch), 8):
                                        kgrp = kch[kb:kb + 8]
                                        ps = pt.tile([128, 1024], F32R, tag="pt")
                                        for j, (ko, ksz) in enumerate(kgrp):
                                            nc.tensor.transpose(
                                                ps[:ksz, j * 128:j * 128 + 128],
                                                xt_in[:, ci, ko:ko + ksz],
                                                identr_sb)
                                        if all(ksz == 128 for _, ksz in kgrp):
                                            _pcopy(
                                                out=xt[:, kb:kb + len(kgrp),
                                                       cc * 128:(cc + 1) * 128],
                                                in_=ps.rearrange(
                                                    "p (k x) -> p k x", x=128
                                                )[:, :len(kgrp), :])
                                        else:
                                            for j, (ko, ksz) in enumerate(kgrp):
                                                _pcopy(
                                                    out=xt[:ksz, kb + j,
                                                           cc * 128:
                                                           (cc + 1) * 128],
                                                    in_=ps[:ksz, j * 128:
                                                           j * 128 + 128])
                            for ni, (no, nsz) in enumerate(nch):
                                for Mo, Msz, dst in ((0, 128, x0t),
                                                     (128, 68, x1t)):
                                    pr = pacc.tile([128, 512], F32, tag="pacc")
                                    ks = nzk[m][0 if Mo == 0 else 1]
                                    for idx, ki in enumerate(ks):
                                        ko, ksz = kch[ki]
                                        nc.tensor.matmul(
                                            pr[:Msz, :nsz],
                                            lhsT=rtt[m][:ksz, ki, Mo:Mo + Msz],
                                            rhs=xt[:ksz, ki, no:no + nsz],
                                            start=(idx == 0),
                                            stop=(idx == len(ks) - 1))
                                    _pcopy(out=dst[:Msz,
                                                   no:no + nsz].bitcast(F32R),
                                           in_=pr[:Msz, :nsz])

                        # zscore along free dim
                        for tl, psz in ((x0t, 128), (x1t, 68)):
                            gs = _chunks(c, 512)
                            st = smalls.tile([128, len(gs), 6], F32, tag="bnst")
                            for gi, (go, gln) in enumerate(gs):
                                nc.vector.bn_stats(out=st[:psz, gi, :],
                                                   in_=tl[:psz, go:go + gln])
                            mv = smalls.tile([128, 2], F32, tag="mv")
                            nc.vector.bn_aggr(out=mv[:psz], in_=st[:psz])
                            sd = smalls.tile([128, 1], F32, tag="sd")
                            nc.scalar.activation(out=sd[:psz], in_=mv[:psz, 1:2],
                                                 func=AF.Sqrt,
                                                 scale=c / (c - 1.0))
                            sc = smalls.tile([128, 1], F32, tag="sc")
                            nc.vector.reciprocal(out=sc[:psz], in_=sd[:psz])
                            bi = smalls.tile([128, 1], F32, tag="bi")
                            nc.vector.tensor_scalar(
                                out=bi[:psz], in0=mv[:psz, 0:1],
                                scalar1=sc[:psz], scalar2=-1.0,
                                op0=ALU.mult, op1=ALU.mult)
                            nc.gpsimd.tensor_scalar(
                                out=tl[:psz].bitcast(F32R), in0=tl[:psz],
                                scalar1=sc[:psz], scalar2=bi[:psz],
                                op0=ALU.mult, op1=ALU.add)

                    # ---- H = sigmoid(W @ Xn) over the sample pair ----
                    hps = hq.tile([16, 2 * WH], F32, tag="hq")
                    for c2 in range(0, cc_n, 2):
                        cn = min(2, cc_n - c2)
                        xn = xnp.tile([128, 2, 2 * WH], F32R, tag="xn")
                        ps = pt.tile([128, 1024], F32R, tag="pt")
                        for ci in range(cn):
                            for si in range(2):
                                off = ci * 512 + si * WH
                                cc = c2 + ci
                                nc.tensor.transpose(
                                    ps[:128, off:off + 128],
                                    xnt0[si][:, cc * 128:
                                             (cc + 1) * 128].bitcast(F32R),
                                    identr_sb)
                                nc.tensor.transpose(
                                    ps[:128, off + 128:off + 196],
                                    xnt1[si][:68, cc * 128:
                                             (cc + 1) * 128].bitcast(F32R),
                                    identr_sb[:68, :68])
                            _pcopy(out=xn[:, ci, :],
                                   in_=ps[:, ci * 512:ci * 512 + 2 * WH])
                        for ci in range(cn):
                            cc = c2 + ci
                            nc.tensor.matmul(
                                hps, lhsT=wtt[m][:, cc, :],
                                rhs=xn[:, ci, :],
                                start=(cc == 0), stop=(cc == cc_n - 1))
                    hsb = hp.tile([16, 2, WH], F32, tag="hsb")
                    nc.scalar.activation(
                        out=hsb.rearrange("p a b -> p (a b)").bitcast(F32R),
                        in_=hps, func=AF.Sigmoid)

                    htbig = {}
                    for si in range(2):
                        b = 4 * si + m
                        bo = 16 * b
                        htb = htp.tile([128, 2, 128], F32, tag="htbig")
                        nc.gpsimd.tensor_copy(
                            out=htb.rearrange("p a b -> p (a b)").bitcast(F32R),
                            in_=zer_sb)
                        pht = pt.tile([128, 1024], F32R, tag="pt")
                        nc.tensor.transpose(pht[:128, 0:Q],
                                            hsb[:Q, si, 0:128].bitcast(F32R),
                                            identr_sb[:Q, :Q])
                        nc.tensor.transpose(pht[:68, Q:2 * Q],
                                            hsb[:Q, si, 128:196].bitcast(F32R),
                                            identr_sb[:Q, :Q])
                        nc.vector.tensor_copy(
                            out=htb[:, 0, bo:bo + Q].bitcast(F32R),
                            in_=pht[:128, 0:Q])
                        nc.vector.tensor_copy(
                            out=htb[:68, 1, bo:bo + Q].bitcast(F32R),
                            in_=pht[:68, Q:2 * Q])
                        htbig[si] = htb
                        # G block accumulation into SBUF accumulator
                        gp = pacc.tile([128, 512], F32, tag="pacc")
                        for wi, wsz in ((0, 128), (1, 68)):
                            nc.tensor.matmul(gp[:, :128],
                                             lhsT=htb[:wsz, wi, :],
                                             rhs=htb[:wsz, wi, :],
                                             start=(wi == 0), stop=(wi == 1))
                        nc.vector.tensor_tensor(out=gsup[spr], in0=gsup[spr],
                                                in1=gp[:, :128], op=ALU.add)
                    # C^T for both blocks of the pair
                    for ni, (no, nsz) in enumerate(nch):
                        cps = pacc.tile([128, 512], F32, tag="pacc")
                        idx = 0
                        for si in range(2):
                            for wi, wsz, xs in ((0, 128, xnt0[si]),
                                                (1, 68, xnt1[si])):
                                nc.tensor.matmul(
                                    cps[:, :nsz],
                                    lhsT=htbig[si][:wsz, wi, :].bitcast(F32R),
                                    rhs=xs[:wsz, no:no + nsz].bitcast(F32R),
                                    start=(idx == 0), stop=(idx == 3))
                                idx += 1
                        _pcopy(out=ct_all[spr][:, OFFS[m] + no:OFFS[m] + no +
                                               nsz].bitcast(F32R),
                               in_=cps[:, :nsz])

                # ---- phase 2 for this super: Newton-Schulz ----
                g = gsup[spr]
                sq = newt.tile([128, 128], F32, tag="sq")
                nc.vector.tensor_mul(sq, g, g)
                rs = newt.tile([128, 1], F32, tag="rs")
                nc.vector.tensor_reduce(out=rs, in_=sq,
                                        axis=mybir.AxisListType.X, op=ALU.add)
                bps = pacc.tile([128, 512], F32, tag="pacc")
                nc.tensor.matmul(bps[:128, 0:1], lhsT=p16_sb, rhs=rs,
                                 start=True, stop=True)
                bf = newt.tile([128, 1], F32, tag="bf")
                nc.scalar.activation(out=bf, in_=bps[:128, 0:1], func=AF.Sqrt)
                al = newt.tile([128, 1], F32, tag="al")
                nc.vector.reciprocal(out=al, in_=bf)
                x_sb = newt.tile([128, 128], F32, tag="xns")
                nc.vector.tensor_scalar_mul(x_sb, ident_sb, al)
                for it in range(NEWTON_ITERS):
                    yps = pacc.tile([128, 512], F32, tag="pacc")
                    nc.tensor.matmul(yps[:128, :128], lhsT=g, rhs=x_sb,
                                     start=True, stop=True)
                    z_sb = newt.tile([128, 128], F32, tag="zns")
                    nc.vector.scalar_tensor_tensor(
                        out=z_sb, in0=ident_sb, scalar=2.0,
                        in1=yps[:128, :128], op0=ALU.mult, op1=ALU.subtract)
                    xps = pacc.tile([128, 512], F32, tag="pacc")
                    nc.tensor.matmul(xps[:128, :128], lhsT=x_sb, rhs=z_sb,
                                     start=True, stop=True)
                    x_new = newt.tile([128, 128], F32, tag="xns")
                    nc.scalar.copy(out=x_new, in_=xps[:128, :128])
                    x_sb = x_new
                mps = pacc.tile([128, 512], F32, tag="pacc")
                nc.tensor.matmul(mps[:128, :128], lhsT=x_sb, rhs=x_sb,
                                 start=True, stop=True)
                m2t = sup.tile([128, 128], F32, tag=f"m2_{spr}",
                               name=f"m2_{spr}")
                nc.vector.tensor_copy(out=m2t.bitcast(F32R),
                                      in_=mps[:128, :128])
                m2_sb[spr] = m2t
                rps = pacc.tile([128, 512], F32, tag="pacc")
                nc.tensor.matmul(rps[:128, 0:1], lhsT=x_sb, rhs=ones_sb,
                                 start=True, stop=True)
                rt_ = sup.tile([128, 1], F32, tag=f"r_{spr}", name=f"r_{spr}")
                nc.vector.tensor_copy(out=rt_, in_=rps[:128, 0:1])
                r_sb[spr] = rt_

            # r4[m][spr] = mask8[m, spr] * r_super(spr)
            r4 = []
            for m in range(4):
                t = smalls.tile([128, 2, 4], F32, tag="r4", name=f"r4_{m}")
                nc.vector.tensor_scalar_mul(t[:, 0, :].bitcast(F32R),
                                            mask8_sb[:, m, 0, :], r_sb[0])
                nc.vector.tensor_scalar_mul(t[:, 1, :].bitcast(F32R),
                                            mask8_sb[:, m, 1, :], r_sb[1])
                r4.append(t)

            # ================= phase 3: variance readout =================
            for m, (c, sp) in enumerate(MEMBERS):
                for ni, (no, nsz) in enumerate(_chunks(c, 512)):
                    g0 = OFFS[m] + no
                    psbs = []
                    for spr in range(2):
                        dfp = pacc.tile([128, 512], F32, tag="pacc")
                        nc.tensor.matmul(
                            dfp[:, :nsz], lhsT=m2_sb[spr].bitcast(F32R),
                            rhs=ct_all[spr][:, g0:g0 + nsz].bitcast(F32R),
                            start=True, stop=True)
                        psb = psbp.tile([128, 512], F32, tag="psb")
                        nc.vector.tensor_mul(psb[:, :nsz].bitcast(F32R),
                                             ct_all[spr][:, g0:g0 + nsz],
                                             dfp[:, :nsz])
                        psbs.append(psb)
                    qps = hq.tile([4, 512], F32, tag="hq")
                    tps = hq.tile([4, 512], F32, tag="hq")
                    for spr in range(2):
                        nc.tensor.matmul(
                            qps[:, :nsz],
                            lhsT=oq8_sb[:, m, spr, :].bitcast(F32R),
                            rhs=psbs[spr][:, :nsz].bitcast(F32R),
                            start=(spr == 0), stop=(spr == 1))
                        nc.tensor.matmul(
                            tps[:, :nsz], lhsT=r4[m][:, spr, :].bitcast(F32R),
                            rhs=ct_all[spr][:, g0:g0 + nsz].bitcast(F32R),
                            start=(spr == 0), stop=(spr == 1))
                    tsq = tsqp.tile([S, 512], F32, tag="tsq")
                    nc.scalar.square(out=tsq[:, :nsz], in_=tps[:S, :nsz])
                    ot = outp.tile([S, 512], F32, tag="out")
                    nc.vector.scalar_tensor_tensor(
                        out=ot[:, :nsz],
                        in0=tsq[:, :nsz], scalar=-1.0 / ((Q - 1) * Q),
                        in1=qps[:S, :nsz], op0=ALU.mult, op1=ALU.add)
                    _dma(out=outd[:, g0:g0 + nsz], in_=ot[:, :nsz])

    nc.finalize()
    return nc


_CACHE = {}


def kernel(x0, x1, x2, x3, W0, W1, W2, W3):
    if "nc" not in _CACHE:
        _CACHE["nc"] = _build_program()
    nc = _CACHE["nc"]
    rts, ident_np, p16_np, mask8_np = _consts()
    xs = [np.asarray(x) for x in (x0, x1, x2, x3)]
    ws = [np.asarray(w) for w in (W0, W1, W2, W3)]
    in_maps = []
    for i in range(NCORES):
        im = {"ident": ident_np, "p16": p16_np, "mask8": mask8_np}
        for m, (c, sp) in enumerate(MEMBERS):
            im[f"x{m}"] = np.ascontiguousarray(
                xs[m][S * i:S * (i + 1)].reshape(S, c, sp * sp), np.float32)
            im[f"wt{m}"] = np.ascontiguousarray(ws[m].T, np.float32)
            if m in rts:
                im[f"rt{m}"] = rts[m]
        in_maps.append(im)
    res = run_bass_kernel_spmd(nc, in_maps, list(range(NCORES)))
    return np.concatenate([r["out"] for r in res.results], axis=0)



# revision 9
# speedup vs baseline: 1.2711x; 1.2711x over previous
"""Trainium2 Bass kernel for nn_ELM_AE_FatSpectral_Ensemble.

Data-parallel over batch: 4 samples/core on 8 cores. Per (sample, member):
  x is cast fp32->bf16 during the input DMA (SWDGE), transposed to x^T via
  DMA XBAR transpose (no PE transposes of the bulk data), resized on PE in
  bf16 (XrT = Rk^T x^T).  zscore is folded algebraically:
    H   = sigmoid(isd .* resize(Wc @ x))      (Wc = W - rowmean(W), host-prep)
    C^T = (isd.*Hblk)^T XrT - (H^T (isd.*mu)) 1^T   (correction via an
          appended -1 row on XrT and a corr row on the scaled H)
  G = H H^T accumulated for 8 (sample, member) blocks directly in PSUM as a
  128x128 block-diagonal supermatrix per sample pair; G^-1 via Newton-Schulz
  (fp32), interleaved at emission with the next super's member processing.
  Variance readout: out = quad/(Q-1) - t^2/(Q(Q-1)) with
  quad_i = C_i G^-2 C_i^T, t_i = C_i (G^-1 1).
"""

import numpy as np
import ml_dtypes

import concourse.bacc as bacc
import concourse.tile as tile
from concourse import mybir
from concourse.bass_utils import run_bass_kernel_spmd

F32 = mybir.dt.float32
F32R = mybir.dt.float32r
F16 = mybir.dt.float16
AF = mybir.ActivationFunctionType
ALU = mybir.AluOpType
BF = np.float16

S = 4
NCORES = 8
SP = 14
WH = SP * SP
Q = 16
MEMBERS = [(256, 56), (512, 28), (1024, 14), (2048, 7)]
OFFS = [0, 256, 768, 1792]
DTOT = 3840
ORDER = [3, 2, 1, 0]          # member processing order (smallest load first)
NEWTON_ITERS = 10


def _weight_mat(n_in, n_out):
    scale = n_out / n_in
    kernel_scale = max(1.0, 1.0 / scale)
    sample_f = (np.arange(n_out) + 0.5) / scale - 0.5
    x = np.abs(sample_f[:, None] - np.arange(n_in)[None, :]) / kernel_scale
    w = np.maximum(0.0, 1.0 - x)
    total = w.sum(axis=1, keepdims=True)
    return (w / np.where(total > 0, total, 1)).astype(np.float32)


def _chunks(n, sz=128):
    return [(i, min(sz, n - i)) for i in range(0, n, sz)]


def _consts():
    """rts_f32 (for nzk), rtt bf16 [128, nk, 196], ident f32, identb bf16,
    p16 f32, mask8 f32."""
    rts = {}
    for m, (c, sp) in enumerate(MEMBERS):
        if sp == SP:
            continue
        R = _weight_mat(sp, SP)
        rt = np.kron(R, R).T.astype(np.float32)   # [uv, 196]
        pad = (-rt.shape[0]) % 128
        if pad:
            rt = np.concatenate([rt, np.zeros((pad, WH), np.float32)], 0)
        rts[m] = np.ascontiguousarray(rt)
    rtb = {m: np.ascontiguousarray(
        rts[m].reshape(-1, 128, WH).transpose(1, 0, 2).astype(BF))
        for m in rts}
    ident = np.eye(128, dtype=np.float32)
    identb = np.eye(128, dtype=BF)
    p16 = np.kron(np.eye(8, dtype=np.float32), np.ones((16, 16), np.float32))
    mask8 = np.zeros((4, 2, 128, 2), np.float32)
    for m in range(4):
        for spr in range(2):
            for si in range(2):
                b = 4 * si + m
                mask8[m, spr, 16 * b:16 * b + 16, si] = 1.0
    mask8 = np.ascontiguousarray(mask8.transpose(2, 0, 1, 3))  # [128, 4, 2, 2]
    return rts, rtb, ident, identb, p16, mask8


def _drive(gen, n):
    if gen is None:
        return False
    for _ in range(n):
        try:
            next(gen)
        except StopIteration:
            return False
    return True


def _build_program():
    rts, rtb, ident_np, identb_np, p16_np, mask8_np = _consts()

    nc = bacc.Bacc()
    xin, waugd, rtd = {}, {}, {}
    for m, (c, sp) in enumerate(MEMBERS):
        uv = sp * sp
        ccn = c // 128
        xin[m] = nc.dram_tensor(f"x{m}", [S, c, uv], F32R, kind="ExternalInput")
        waugd[m] = nc.dram_tensor(f"waug{m}", [128, ccn, Q], F16,
                                  kind="ExternalInput")
        if m in rtb:
            rtd[m] = nc.dram_tensor(f"rt{m}", list(rtb[m].shape), F16,
                                    kind="ExternalInput")
    identd = nc.dram_tensor("ident", [128, 128], F32, kind="ExternalInput")
    identbd = nc.dram_tensor("identb", [128, 128], F16, kind="ExternalInput")
    p16d = nc.dram_tensor("p16", [128, 128], F32, kind="ExternalInput")
    mask8d = nc.dram_tensor("mask8", [128, 4, 2, 2], F32, kind="ExternalInput")
    outd = nc.dram_tensor("out", [S, DTOT], F32, kind="ExternalOutput")

    # nonzero k-chunk lists per (m, Mblock)
    nzk = {}
    for m in rts:
        uv = MEMBERS[m][1] ** 2
        nzk[m] = {}
        for Mi, (Mo, Msz) in enumerate([(0, 128), (128, 68)]):
            nzk[m][Mi] = [ki for ki, (ko, ksz) in enumerate(_chunks(uv))
                          if np.any(rts[m][ko:ko + ksz, Mo:Mo + Msz] != 0)]

    from contextlib import ExitStack
    _ceng = [0]

    def _pcopy(out, in_):
        _ceng[0] ^= 1
        if _ceng[0]:
            nc.scalar.copy(out=out, in_=in_)
        else:
            nc.vector.tensor_copy(out=out, in_=in_)

    with tile.TileContext(nc) as tc, ExitStack() as _es:
        _p = lambda **kw: _es.enter_context(tc.tile_pool(**kw))
        consts = _p(name="consts", bufs=1)
        xbfp = _p(name="xbfp", bufs=1)
        xtp = _p(name="xtp", bufs=1)
        xrtp = _p(name="xrtp", bufs=1)
        pstp = _p(name="pstp", bufs=1)
        pctp = _p(name="pctp", bufs=1)
        hp = _p(name="hp", bufs=2)
        smalls = _p(name="smalls", bufs=3)
        sup = _p(name="sup", bufs=1)
        newt = _p(name="newt", bufs=2)
        outp = _p(name="outp", bufs=2)
        pacc = _p(name="pacc", bufs=2, space="PSUM")   # 2 banks
        pct = _p(name="pct", bufs=2, space="PSUM")     # 2 banks
        pg = _p(name="pg", bufs=1, space="PSUM")       # 1 bank
        ps3 = _p(name="ps3", bufs=3, space="PSUM")     # 3 banks, shared tag

        # ---------------- constants ----------------
        ident_sb = consts.tile([128, 128], F32, tag="ident")
        nc.sync.dma_start(out=ident_sb, in_=identd[:, :])
        identb_sb = consts.tile([128, 128], F16, tag="identb")
        nc.sync.dma_start(out=identb_sb, in_=identbd[:, :])
        p16_sb = consts.tile([128, 128], F32, tag="p16")
        nc.sync.dma_start(out=p16_sb, in_=p16d[:, :])
        mask8_sb = consts.tile([128, 4, 2, 2], F32, tag="mask8")
        nc.sync.dma_start(out=mask8_sb, in_=mask8d[:, :, :, :])
        oq8_sb = consts.tile([128, 4, 2, 2], F16, tag="oq8")
        nc.vector.tensor_scalar_mul(
            oq8_sb.rearrange("p a b c -> p (a b c)"),
            mask8_sb.rearrange("p a b c -> p (a b c)"), 1.0 / (Q - 1))
        ones_sb = consts.tile([128, 1], F32, tag="ones")
        nc.vector.memset(ones_sb, 1.0)

        rtt, waug_sb = {}, {}
        for m, (c, sp) in enumerate(MEMBERS):
            ccn = c // 128
            waug_sb[m] = consts.tile([128, ccn, Q], F16, tag=f"waug{m}",
                                     name=f"waug{m}")
            nc.sync.dma_start(out=waug_sb[m], in_=waugd[m][:, :, :])
            if m in rtb:
                nk = rtb[m].shape[1]
                rtt[m] = consts.tile([128, nk, WH], F16, tag=f"rt{m}",
                                     name=f"rt{m}")
                nc.sync.dma_start(out=rtt[m], in_=rtd[m][:, :, :])

        ct_all = []
        for spr in range(2):
            t = sup.tile([128, DTOT], F16, tag=f"ct{spr}", name=f"ct{spr}")
            ct_all.append(t)
        g_sb = [None, None]
        m2_sb = [None, None]
        r_sb = [None, None]
        r4_all = [[None] * 4, [None] * 4]

        # ================= per-super member processing =================
        def member(spr, m, xbf, g_ps, gfirst, glast):
            c, sp = MEMBERS[m]
            uv = sp * sp
            ccn = c // 128
            kch = _chunks(uv)
            nk = len(kch)
            uvp = nk * 128

            # ---- x^T via DMA XBAR transpose (bf16) ----
            xts = {}
            for si in range(2):
                xt = xtp.tile([128, ccn, nk, 128], F16, tag=f"xt{m}_{si}",
                              name=f"xt{m}_{si}")
                nc.sync.dma_start(out=xt,
                                  in_=xbf[si].rearrange("p a b -> p (a b)"),
                                  transpose=True)
                xts[si] = xt

            # ---- P = Wc @ x (bf16, natural layout) ----
            pst = pstp.tile([64, uvp], F16, tag=f"pst{m}", name=f"pst{m}")
            nc.vector.memset(pst, 0.0)
            for si in range(2):
                for no, nsz in _chunks(uv, 512):
                    pp_ps = ps3.tile([16, 512], F32, tag="ps")
                    for cc in range(ccn):
                        nc.tensor.matmul(
                            pp_ps[:, :nsz], lhsT=waug_sb[m][:, cc, :],
                            rhs=xbf[si][:, cc, no:no + nsz],
                            start=(cc == 0), stop=(cc == ccn - 1))
                    _pcopy(out=pst[32 * si:32 * si + 16, no:no + nsz],
                           in_=pp_ps[:Q, :nsz])

            # ---- Pc^T via PE transposes (small) ----
            pct_t = pctp.tile([128, nk, 64], F16, tag=f"pct{m}",
                              name=f"pct{m}")
            for ki in range(nk):
                tp_ps = ps3.tile([128, 64], F16, tag="ps")
                nc.tensor.transpose(tp_ps[:, :],
                                    pst[:64, ki * 128:(ki + 1) * 128],
                                    identb_sb[:64, :64])
                _pcopy(out=pct_t[:, ki, :], in_=tp_ps)

            # ---- resize: XrT = Rk^T x^T (bf16), skip for m == 2 ----
            xrt = {}   # (si, wi) -> (tile, psz)
            if m != 2:
                for si in range(2):
                    x0t = xrtp.tile([128, c], F16, tag=f"x0t{m}_{si}",
                                    name=f"x0t{m}_{si}")
                    x1t = xrtp.tile([97, c], F16, tag=f"x1t{m}_{si}",
                                    name=f"x1t{m}_{si}")
                    nc.vector.memset(x1t[64:97, :], 0.0)
                    nc.vector.memset(x1t[96:97, :], -1.0)
                    for wi, (Mo, Msz, dst) in enumerate(
                            ((0, 128, x0t), (128, 68, x1t))):
                        for no, nsz in _chunks(c, 512):
                            cc0 = no // 128
                            ncc = (nsz + 127) // 128
                            pr = pacc.tile([128, 512], F32, tag="pacc")
                            ks = nzk[m][wi]
                            for idx, ki in enumerate(ks):
                                ko, ksz = kch[ki]
                                nc.tensor.matmul(
                                    pr[:Msz, :nsz],
                                    lhsT=rtt[m][:ksz, ki, Mo:Mo + Msz],
                                    rhs=xts[si][:ksz, cc0:cc0 + ncc, ki, :],
                                    start=(idx == 0),
                                    stop=(idx == len(ks) - 1))
                            _pcopy(out=dst[:Msz, no:no + nsz],
                                   in_=pr[:Msz, :nsz])
                    xrt[si, 0] = (x0t, 128)
                    xrt[si, 1] = (x1t, 68)
            else:
                for si in range(2):
                    nc.vector.memset(xts[si][96:97, :, 1, :], -1.0)

            def _xview(si, wi, no, nsz):
                hi = 128 if wi == 0 else 97
                if m != 2:
                    t, psz = xrt[si, wi]
                    return t[:hi, no:no + nsz]
                cc0 = no // 128
                ncc = (nsz + 127) // 128
                return xts[si][:hi, cc0:cc0 + ncc, wi, :]

            # ---- stats per (si, wi): mu, isd, v = isd*mu ----
            stats = {}
            for si in range(2):
                for wi, psz in ((0, 128), (1, 68)):
                    if m != 2:
                        t, _ = xrt[si, wi]
                        srcs = [t[:psz, go:go + gln]
                                for go, gln in _chunks(c, 512)]
                    else:
                        srcs = [xts[si][:psz, cc, wi, :]
                                for cc in range(ccn)]
                    st = smalls.tile([128, len(srcs), 6], F32, tag="bnst")
                    for gi, src in enumerate(srcs):
                        nc.vector.bn_stats(out=st[:psz, gi, :], in_=src)
                    mv = smalls.tile([128, 2], F32, tag="mv")
                    nc.vector.bn_aggr(out=mv[:psz], in_=st[:psz])
                    sd = smalls.tile([128, 1], F32, tag="sd")
                    nc.scalar.activation(out=sd[:psz], in_=mv[:psz, 1:2],
                                         func=AF.Sqrt, scale=c / (c - 1.0))
                    isd = smalls.tile([128, 1], F32, tag="isd")
                    nc.vector.reciprocal(out=isd[:psz], in_=sd[:psz])
                    v = smalls.tile([128, 1], F16, tag="vv")
                    nc.vector.tensor_mul(v[:psz], mv[:psz, 0:1], isd[:psz])
                    stats[si, wi] = (isd, v)

            # ---- PRc^T = Rk^T Pc^T (fp32 PSUM), then sigmoid -> H^T ----
            prc = {}
            if m != 2:
                for wi, (Mo, Msz) in enumerate(((0, 128), (128, 68))):
                    ps = ps3.tile([128, 64], F32, tag="ps")
                    ks = nzk[m][wi]
                    for idx, ki in enumerate(ks):
                        ko, ksz = kch[ki]
                        nc.tensor.matmul(
                            ps[:Msz, :], lhsT=rtt[m][:ksz, ki, Mo:Mo + Msz],
                            rhs=pct_t[:ksz, ki, :],
                            start=(idx == 0), stop=(idx == len(ks) - 1))
                    prc[wi] = ps

            hts = {}
            for si in range(2):
                h0 = hp.tile([128, 128], F16, tag=f"h0_{si}", name=f"h0_{si}")
                h1 = hp.tile([69, 128], F16, tag=f"h1_{si}", name=f"h1_{si}")
                nc.gpsimd.memset(h0, 0.0)
                nc.gpsimd.memset(h1, 0.0)
                bo = 16 * (4 * si + m)
                for wi, Msz in ((0, 128), (1, 68)):
                    isd, _ = stats[si, wi]
                    if m != 2:
                        src = prc[wi][:Msz, 32 * si:32 * si + 16]
                    else:
                        src = pct_t[:Msz, wi, 32 * si:32 * si + 16]
                    dst = (h0 if wi == 0 else h1)[:Msz, bo:bo + Q]
                    nc.scalar.activation(out=dst, in_=src, func=AF.Sigmoid,
                                         scale=isd[:Msz])
                hts[si] = (h0, h1)

            # ---- G supermatrix accumulation (PSUM, whole super) ----
            for si in range(2):
                h0, h1 = hts[si]
                nc.tensor.matmul(g_ps, lhsT=h0, rhs=h0,
                                 start=(gfirst and si == 0), stop=False)
                nc.tensor.matmul(g_ps, lhsT=h1[:68, :], rhs=h1[:68, :],
                                 start=False, stop=(glast and si == 1))

            # ---- corr row + isd-scaled H (for C^T fold) ----
            hss = {}
            for si in range(2):
                h0, h1 = hts[si]
                isd0, v0 = stats[si, 0]
                isd1, v1 = stats[si, 1]
                cr = ps3.tile([1, 128], F32, tag="ps")
                nc.tensor.matmul(cr[:1, :], lhsT=v0, rhs=h0,
                                 start=True, stop=False)
                nc.tensor.matmul(cr[:1, :], lhsT=v1[:68], rhs=h1[:68, :],
                                 start=False, stop=True)
                hs0 = hp.tile([128, 128], F16, tag=f"hs0_{si}",
                              name=f"hs0_{si}")
                hs1 = hp.tile([97, 128], F16, tag=f"hs1_{si}",
                              name=f"hs1_{si}")
                nc.gpsimd.memset(hs1[64:97, :], 0.0)
                nc.vector.tensor_scalar_mul(hs0, h0, isd0)
                nc.gpsimd.tensor_scalar_mul(hs1[:68, :], h1[:68, :],
                                            isd1[:68])
                nc.vector.tensor_copy(out=hs1[96:97, :], in_=cr)
                hss[si] = (hs0, hs1)

            # ---- C^T into ct_all supermatrix ----
            for no, nsz in _chunks(c, 512):
                ct_ps = pct.tile([128, 512], F32, tag="pct")
                idx = 0
                for si in range(2):
                    hs0, hs1 = hss[si]
                    for wi in range(2):
                        lhsT = hs0 if wi == 0 else hs1
                        nc.tensor.matmul(ct_ps[:, :nsz], lhsT=lhsT,
                                         rhs=_xview(si, wi, no, nsz),
                                         start=(idx == 0), stop=(idx == 3))
                        idx += 1
                _pcopy(out=ct_all[spr][:, OFFS[m] + no:OFFS[m] + no + nsz],
                       in_=ct_ps[:, :nsz])

        # ---------------- Newton-Schulz generator ----------------
        def ns_gen(spr, g_ps):
            g = sup.tile([128, 128], F32, tag=f"g{spr}", name=f"g{spr}")
            nc.vector.tensor_copy(out=g, in_=g_ps)
            g_sb[spr] = g
            sq = newt.tile([128, 128], F32, tag="sq")
            nc.vector.tensor_mul(sq, g, g)
            rs = newt.tile([128, 1], F32, tag="rs")
            nc.vector.tensor_reduce(out=rs, in_=sq,
                                    axis=mybir.AxisListType.X, op=ALU.add)
            bps = ps3.tile([128, 32], F32, tag="ps")
            nc.tensor.matmul(bps[:128, 0:1], lhsT=p16_sb, rhs=rs,
                             start=True, stop=True)
            bf = newt.tile([128, 1], F32, tag="bf")
            nc.scalar.activation(out=bf, in_=bps[:128, 0:1], func=AF.Sqrt)
            al = newt.tile([128, 1], F32, tag="al")
            nc.vector.reciprocal(out=al, in_=bf)
            x_sb = newt.tile([128, 128], F32, tag="xns")
            nc.vector.tensor_scalar_mul(x_sb, ident_sb, al)
            yield
            for it in range(NEWTON_ITERS):
                yps = pacc.tile([128, 512], F32, tag="pacc")
                nc.tensor.matmul(yps[:128, :128], lhsT=g, rhs=x_sb,
                                 start=True, stop=True)
                z_sb = newt.tile([128, 128], F32, tag="zns")
                nc.vector.scalar_tensor_tensor(
                    out=z_sb, in0=ident_sb, scalar=2.0,
                    in1=yps[:128, :128], op0=ALU.mult, op1=ALU.subtract)
                xps = pacc.tile([128, 512], F32, tag="pacc")
                nc.tensor.matmul(xps[:128, :128], lhsT=x_sb, rhs=z_sb,
                                 start=True, stop=True)
                x_new = newt.tile([128, 128], F32, tag="xns")
                nc.scalar.copy(out=x_new, in_=xps[:128, :128])
                x_sb = x_new
                yield
            mps = pacc.tile([128, 512], F32, tag="pacc")
            nc.tensor.matmul(mps[:128, :128], lhsT=x_sb, rhs=x_sb,
                             start=True, stop=True)
            m2t = sup.tile([128, 128], F16, tag=f"m2_{spr}", name=f"m2_{spr}")
            nc.vector.tensor_copy(out=m2t, in_=mps[:128, :128])
            m2_sb[spr] = m2t
            rps = ps3.tile([128, 32], F32, tag="ps")
            nc.tensor.matmul(rps[:128, 0:1], lhsT=x_sb, rhs=ones_sb,
                             start=True, stop=True)
            rt_ = sup.tile([128, 1], F32, tag=f"r_{spr}", name=f"r_{spr}")
            nc.vector.tensor_copy(out=rt_, in_=rps[:128, 0:1])
            r_sb[spr] = rt_
            yield
            for m in range(4):
                t = sup.tile([128, 2], F16, tag=f"r4_{spr}_{m}",
                             name=f"r4_{spr}_{m}")
                nc.vector.tensor_scalar_mul(t, mask8_sb[:, m, spr, :], rt_)
                r4_all[spr][m] = t
            yield

        # ---------------- phase-3 generator (per super) ----------------
        def ph3_gen(spr):
            for m, (c, sp) in enumerate(MEMBERS):
                for no, nsz in _chunks(c, 512):
                    g0 = OFFS[m] + no
                    dfp = pacc.tile([128, 512], F32, tag="pacc")
                    nc.tensor.matmul(
                        dfp[:, :nsz], lhsT=m2_sb[spr],
                        rhs=ct_all[spr][:, g0:g0 + nsz],
                        start=True, stop=True)
                    psb = outp.tile([128, 512], F16, tag="psb")
                    nc.vector.tensor_mul(psb[:, :nsz],
                                         ct_all[spr][:, g0:g0 + nsz],
                                         dfp[:, :nsz])
                    qps = ps3.tile([16, 512], F32, tag="ps")
                    nc.tensor.matmul(
                        qps[:2, :nsz],
                        lhsT=oq8_sb[:, m, spr, :],
                        rhs=psb[:, :nsz],
                        start=True, stop=True)
                    tps = ps3.tile([16, 512], F32, tag="ps")
                    nc.tensor.matmul(
                        tps[:2, :nsz],
                        lhsT=r4_all[spr][m],
                        rhs=ct_all[spr][:, g0:g0 + nsz],
                        start=True, stop=True)
                    ot = outp.tile([2, 512], F32, tag="ot")
                    nc.scalar.square(out=ot[:, :nsz], in_=tps[:2, :nsz])
                    nc.vector.scalar_tensor_tensor(
                        out=ot[:, :nsz],
                        in0=ot[:, :nsz], scalar=-1.0 / ((Q - 1) * Q),
                        in1=qps[:2, :nsz], op0=ALU.mult, op1=ALU.add)
                    nc.scalar.dma_start(
                        out=outd[2 * spr:2 * spr + 2, g0:g0 + nsz],
                        in_=ot[:, :nsz])
                    yield

        # ================= driver =================
        gens = [None, None]
        ph3_0 = None
        for spr in range(2):
            # issue all cast-DMAs for this super up-front (SWDGE casts)
            xbf_all = {}
            for m in ORDER:
                c, sp = MEMBERS[m]
                uv = sp * sp
                ccn = c // 128
                nk = (uv + 127) // 128
                uvp = nk * 128
                for si in range(2):
                    s = 2 * spr + si
                    t = xbfp.tile([128, ccn, uvp], F16, tag=f"xbf{m}_{si}",
                                  name=f"xbf{m}_{si}")
                    nc.gpsimd.dma_start(
                        out=t[:, :, :uv],
                        in_=xin[m][s, :, :].rearrange("(k p) v -> p k v",
                                                      p=128))
                    if uv < uvp:
                        nc.gpsimd.memset(t[:, :, uv:uvp], 0.0)
                    xbf_all[m, si] = t
            g_ps = pg.tile([128, 128], F32, tag="pg")
            for mi, m in enumerate(ORDER):
                if spr == 1:
                    _drive(gens[0], 4)
                    if mi == 3 and ph3_0 is None and r4_all[0][0] is not None:
                        ph3_0 = ph3_gen(0)
                    _drive(ph3_0, 2)
                member(spr, m, (xbf_all[m, 0], xbf_all[m, 1]), g_ps,
                       gfirst=(mi == 0), glast=(mi == 3))
            gens[spr] = ns_gen(spr, g_ps)
            _drive(gens[spr], 1)

        # tail: finish NS0, interleave NS1 with phase3(0), then phase3(1)
        while _drive(gens[0], 1):
            pass
        if ph3_0 is None:
            ph3_0 = ph3_gen(0)
        more1 = True
        more0 = True
        while more0 or more1:
            more1 = _drive(gens[1], 1)
            more0 = _drive(ph3_0, 1)
        for _ in ph3_gen(1):
            pass

    nc.finalize()
    return nc


def _in_maps(xs, ws):
    """xs: list of 4 np arrays [32, c, h, w] fp32; ws: list of [Q, c]."""
    rts, rtb, ident_np, identb_np, p16_np, mask8_np = _consts()
    waug = {}
    for m, (c, sp) in enumerate(MEMBERS):
        W = np.asarray(ws[m], np.float32)
        Wc = W - W.sum(axis=1, keepdims=True) / c
        wa = np.ascontiguousarray(
            Wc.T.reshape(c // 128, 128, Q).transpose(1, 0, 2).astype(BF))
        waug[m] = wa
    in_maps = []
    for i in range(NCORES):
        im = {"ident": ident_np, "identb": identb_np, "p16": p16_np,
              "mask8": mask8_np}
        for m, (c, sp) in enumerate(MEMBERS):
            im[f"x{m}"] = np.ascontiguousarray(
                xs[m][S * i:S * (i + 1)].reshape(S, c, sp * sp), np.float32)
            im[f"waug{m}"] = waug[m]
            if m in rtb:
                im[f"rt{m}"] = rtb[m]
        in_maps.append(im)
    return in_maps


_CACHE = {}


def kernel(x0, x1, x2, x3, W0, W1, W2, W3):
    if "nc" not in _CACHE:
        _CACHE["nc"] = _build_program()
    nc = _CACHE["nc"]
    xs = [np.asarray(x) for x in (x0, x1, x2, x3)]
    ws = [np.asarray(w) for w in (W0, W1, W2, W3)]
    in_maps = _in_maps(xs, ws)
    res = run_bass_kernel_spmd(nc, in_maps, list(range(NCORES)))
    return np.concatenate([r["out"] for r in res.results], axis=0)
